# revision 6
# baseline (speedup 1.0000x reference)
"""Trainium2 Bass kernel for DifferentiablePortfolioSim.

Computes, for allocations/returns of shape [B, T, A] = [1024, 2048, 64]:
    port_return[b,t] = sum_a alloc[b,t,a] * ret[b,t,a]
    turnover[b,t]    = sum_a |alloc[b,t,a] - alloc[b,t-1,a]|   (alloc[:,-1]=0)
    net_return       = port_return - 0.001 * turnover
    equity_curve     = [1, cumprod_t(1 + net_return)]          # [B, T+1]
Returns (equity_curve, net_return).

Sharding: data parallel over batch, 128 rows per core on 8 cores; batch rows
on the 128 SBUF partitions, time*assets streamed on the free dim in chunks.

Inputs are pre-cast to fp16 on the host: halves HBM traffic (the memory
roofline) and enables the DVE 2x perf mode for the elementwise passes.
Since equity decays exponentially (mean net return is negative),
absmax-relative error stays ~1e-4.

Engine split per chunk (DVE is the measured bottleneck, ~0.52ns/elem at 2x;
GPSIMD ~3.2ns/elem; ACT ~0.9ns/elem 1-input only):
  - DVE:  fp16 product into the low half of a combo tile, the first
          SUB_DVE timesteps of the shifted diff, and one pairwise-add
          reduction ladder over the combo tile (TensorReduce has no DVE
          perf modes, a ladder of fp16 2x adds is ~2x faster).
  - ACT:  elementwise |diff| into the high half of the combo tile
  - GPSIMD: the remaining timesteps of the shifted diff
The ladder output interleaves port/turn per chunk in one persistent tile;
the tail un-interleaves via strided access patterns.
"""

import numpy as np

B, T, A = 1024, 2048, 64
NCORES = 8
BP = B // NCORES  # 128 batch rows per core == SBUF partitions
TC = 64           # timesteps per chunk
NCH = T // TC
SUB_DVE = 26      # timesteps of the diff pass on DVE; rest on GPSIMD

TRANSACTION_COST = 0.001

_compiled = None
LAST_RESULTS = None


def _build():
    import concourse.mybir as mybir
    from concourse import bacc
    from concourse.tile import TileContext

    f32 = mybir.dt.float32
    f16 = mybir.dt.float16
    Alu = mybir.AluOpType

    nc = bacc.Bacc(
        "TRN2",
        debug=False,
        target_bir_lowering=False,
        num_devices=NCORES,
    )

    a_in = nc.dram_tensor("alloc", [BP, T * A], f16, kind="ExternalInput").ap()
    r_in = nc.dram_tensor("ret", [BP, T * A], f16, kind="ExternalInput").ap()
    eq_out = nc.dram_tensor("equity", [BP, T + 1], f32, kind="ExternalOutput").ap()
    net_out = nc.dram_tensor("net", [BP, T], f32, kind="ExternalOutput").ap()

    with TileContext(nc) as tc:
        with (
            tc.tile_pool(name="persist", bufs=1) as pp,
            tc.tile_pool(name="dma", bufs=3) as dp,
            tc.tile_pool(name="chunk", bufs=3) as cp,
        ):
            # pt interleaves [port(TC) | turn(TC)] per chunk
            pt = pp.tile([BP, 2 * T], f32, tag="pt")
            net = pp.tile([BP, T], f32, tag="net")
            g = pp.tile([BP, T], f32, tag="g")
            eq = pp.tile([BP, T + 1], f32, tag="eq")

            for k in range(NCH):
                t0 = k * TC
                # a_t holds TC+1 timesteps: one lookback step + the chunk.
                a_t = dp.tile([BP, (TC + 1) * A], f16, tag="a")
                r_t = dp.tile([BP, TC * A], f16, tag="r")
                dif = cp.tile([BP, TC * A], f16, tag="dif")
                # combo: [ prod (TC*A) | |dif| (TC*A) ]
                combo = cp.tile([BP, 2 * TC * A], f16, tag="combo")

                if k == 0:
                    # prev_alloc at t=0 is zeros
                    nc.vector.memset(a_t[:, 0:A], 0.0)
                    nc.sync.dma_start(out=a_t[:, A:], in_=a_in[:, 0 : TC * A])
                else:
                    nc.sync.dma_start(
                        out=a_t[:], in_=a_in[:, (t0 - 1) * A : (t0 + TC) * A]
                    )
                nc.sync.dma_start(out=r_t[:], in_=r_in[:, t0 * A : (t0 + TC) * A])

                # DVE: full shifted diff first so ACT's abs can start early
                nc.vector.tensor_sub(
                    out=dif[:], in0=a_t[:, A:], in1=a_t[:, 0 : TC * A]
                )

                # ACT: |diff| into high half of combo
                nc.scalar.activation(
                    out=combo[:, TC * A :],
                    in_=dif[:],
                    func=mybir.ActivationFunctionType.Abs,
                )

                # fp16 product into low half of combo, split DVE / GPSIMD
                # (the product feeds the ladder directly, so GPSIMD's slow
                # share does not add an ACT hop to the chain)
                ne = SUB_DVE * A
                nc.vector.tensor_mul(
                    out=combo[:, 0:ne], in0=a_t[:, A : A + ne], in1=r_t[:, 0:ne]
                )
                nc.gpsimd.tensor_mul(
                    out=combo[:, ne : TC * A],
                    in0=a_t[:, A + ne :],
                    in1=r_t[:, ne:],
                )

                # single pairwise-add ladder over both halves:
                # 2*TC segments of length A -> one sum each
                nseg = 2 * TC
                cur = combo[:]
                width = A
                lvl = 0
                while width > 2:
                    width //= 2
                    nxt = cp.tile([BP, nseg * width], f16, tag=f"l{lvl}")
                    c3 = cur.rearrange("p (t a) -> p t a", a=2 * width)
                    nc.vector.tensor_add(
                        out=nxt[:],
                        in0=c3[:, :, 0:width],
                        in1=c3[:, :, width : 2 * width],
                    )
                    cur = nxt[:]
                    lvl += 1
                c3 = cur.rearrange("p (t a) -> p t a", a=2)
                nc.vector.tensor_add(
                    out=pt[:, k * nseg : (k + 1) * nseg],
                    in0=c3[:, :, 0:1],
                    in1=c3[:, :, 1:2],
                )

            # un-interleave and combine: net = port - 0.001 * turn
            pt3 = pt[:].rearrange("p (k d) -> p k d", d=2 * TC)
            nc.vector.scalar_tensor_tensor(
                out=net[:].rearrange("p (k d) -> p k d", d=TC),
                in0=pt3[:, :, TC : 2 * TC],
                scalar=-TRANSACTION_COST,
                in1=pt3[:, :, 0:TC],
                op0=Alu.mult,
                op1=Alu.add,
            )
            # g = 1 + net
            nc.vector.tensor_scalar_add(out=g[:], in0=net[:], scalar1=1.0)
            # equity: eq[0] = 1, eq[1:] = cumprod(g)
            nc.vector.memset(eq[:, 0:1], 1.0)
            nc.vector.tensor_tensor_scan(
                out=eq[:, 1 : T + 1],
                data0=g[:],
                data1=g[:],
                initial=1.0,
                op0=Alu.mult,
                op1=Alu.bypass,
            )

            nc.sync.dma_start(out=net_out[:], in_=net[:])
            nc.sync.dma_start(out=eq_out[:], in_=eq[:])

    nc.compile()
    return nc


def _get_compiled():
    global _compiled
    if _compiled is None:
        _compiled = _build()
    return _compiled


def kernel(allocations, returns):
    global LAST_RESULTS
    from concourse.bass_utils import run_bass_kernel_spmd

    nc = _get_compiled()

    a = np.asarray(allocations, dtype=np.float32).astype(np.float16).reshape(B, T * A)
    r = np.asarray(returns, dtype=np.float32).astype(np.float16).reshape(B, T * A)

    in_maps = [
        {"alloc": a[i * BP : (i + 1) * BP], "ret": r[i * BP : (i + 1) * BP]}
        for i in range(NCORES)
    ]
    res = run_bass_kernel_spmd(nc, in_maps, core_ids=list(range(NCORES)))
    LAST_RESULTS = res

    equity = np.concatenate([res.results[i]["equity"] for i in range(NCORES)], axis=0)
    net = np.concatenate([res.results[i]["net"] for i in range(NCORES)], axis=0)
    return equity, net


# revision 7
# speedup vs baseline: 1.0012x; 1.0012x over previous
"""Trainium2 Bass kernel for DifferentiablePortfolioSim.

Computes, for allocations/returns of shape [B, T, A] = [1024, 2048, 64]:
    port_return[b,t] = sum_a alloc[b,t,a] * ret[b,t,a]
    turnover[b,t]    = sum_a |alloc[b,t,a] - alloc[b,t-1,a]|   (alloc[:,-1]=0)
    net_return       = port_return - 0.001 * turnover
    equity_curve     = [1, cumprod_t(1 + net_return)]          # [B, T+1]
Returns (equity_curve, net_return).

Sharding: data parallel over batch, 128 rows per core on 8 cores; batch rows
on the 128 SBUF partitions, time*assets streamed on the free dim in chunks.

Inputs are pre-cast to fp16 on the host: halves HBM traffic (the memory
roofline) and enables the DVE 2x perf mode for the elementwise passes.
Since equity decays exponentially (mean net return is negative),
absmax-relative error stays ~1e-4.

Engine split per chunk (DVE is the measured bottleneck, ~0.52ns/elem at 2x;
GPSIMD ~3.2ns/elem; ACT ~0.9ns/elem 1-input only):
  - DVE:  fp16 product into the low half of a combo tile, the first
          SUB_DVE timesteps of the shifted diff, and one pairwise-add
          reduction ladder over the combo tile (TensorReduce has no DVE
          perf modes, a ladder of fp16 2x adds is ~2x faster).
  - ACT:  elementwise |diff| into the high half of the combo tile
  - GPSIMD: the remaining timesteps of the shifted diff
The ladder output interleaves port/turn per chunk in one persistent tile;
the tail un-interleaves via strided access patterns.
"""

import numpy as np

B, T, A = 1024, 2048, 64
NCORES = 8
BP = B // NCORES  # 128 batch rows per core == SBUF partitions
TC = 64           # timesteps per chunk
NCH = T // TC
SUB_DVE = 26      # timesteps of the diff pass on DVE; rest on GPSIMD

TRANSACTION_COST = 0.001

_compiled = None
LAST_RESULTS = None


def _build():
    import concourse.mybir as mybir
    from concourse import bacc
    from concourse.tile import TileContext

    f32 = mybir.dt.float32
    f16 = mybir.dt.float16
    Alu = mybir.AluOpType

    nc = bacc.Bacc(
        "TRN2",
        debug=False,
        target_bir_lowering=False,
        num_devices=NCORES,
    )

    a_in = nc.dram_tensor("alloc", [BP, T * A], f16, kind="ExternalInput").ap()
    r_in = nc.dram_tensor("ret", [BP, T * A], f16, kind="ExternalInput").ap()
    eq_out = nc.dram_tensor("equity", [BP, T + 1], f32, kind="ExternalOutput").ap()
    net_out = nc.dram_tensor("net", [BP, T], f32, kind="ExternalOutput").ap()

    with TileContext(nc) as tc:
        with (
            tc.tile_pool(name="persist", bufs=1) as pp,
            tc.tile_pool(name="dma", bufs=3) as dp,
            tc.tile_pool(name="chunk", bufs=3) as cp,
        ):
            # pt interleaves [port(TC) | turn(TC)] per chunk
            pt = pp.tile([BP, 2 * T], f32, tag="pt")
            net = pp.tile([BP, T], f32, tag="net")
            g = pp.tile([BP, T], f32, tag="g")
            eq = pp.tile([BP, T + 1], f32, tag="eq")

            def emit_loads_and_elementwise(k):
                t0 = k * TC
                # a_t holds TC+1 timesteps: one lookback step + the chunk.
                a_t = dp.tile([BP, (TC + 1) * A], f16, tag="a")
                r_t = dp.tile([BP, TC * A], f16, tag="r")
                dif = cp.tile([BP, TC * A], f16, tag="dif")
                # combo: [ prod (TC*A) | |dif| (TC*A) ]
                combo = cp.tile([BP, 2 * TC * A], f16, tag="combo")

                if k == 0:
                    # prev_alloc at t=0 is zeros
                    nc.vector.memset(a_t[:, 0:A], 0.0)
                    nc.sync.dma_start(out=a_t[:, A:], in_=a_in[:, 0 : TC * A])
                else:
                    nc.sync.dma_start(
                        out=a_t[:], in_=a_in[:, (t0 - 1) * A : (t0 + TC) * A]
                    )
                nc.sync.dma_start(out=r_t[:], in_=r_in[:, t0 * A : (t0 + TC) * A])

                # GPSIMD's product share first (it's the slowest producer and
                # only needs the DMAs); it feeds the ladder directly so its
                # latency has no ACT hop
                ne = SUB_DVE * A
                nc.gpsimd.tensor_mul(
                    out=combo[:, ne : TC * A],
                    in0=a_t[:, A + ne :],
                    in1=r_t[:, ne:],
                )

                # DVE: full shifted diff early so ACT's abs can start
                nc.vector.tensor_sub(
                    out=dif[:], in0=a_t[:, A:], in1=a_t[:, 0 : TC * A]
                )

                # ACT: |diff| into high half of combo
                nc.scalar.activation(
                    out=combo[:, TC * A :],
                    in_=dif[:],
                    func=mybir.ActivationFunctionType.Abs,
                )

                # DVE's product share
                nc.vector.tensor_mul(
                    out=combo[:, 0:ne], in0=a_t[:, A : A + ne], in1=r_t[:, 0:ne]
                )
                return combo

            def emit_ladder(k, combo):
                # single pairwise-add ladder over both halves:
                # 2*TC segments of length A -> one sum each
                nseg = 2 * TC
                cur = combo[:]
                width = A
                lvl = 0
                while width > 2:
                    width //= 2
                    nxt = cp.tile([BP, nseg * width], f16, tag=f"l{lvl}")
                    c3 = cur.rearrange("p (t a) -> p t a", a=2 * width)
                    nc.vector.tensor_add(
                        out=nxt[:],
                        in0=c3[:, :, 0:width],
                        in1=c3[:, :, width : 2 * width],
                    )
                    cur = nxt[:]
                    lvl += 1
                c3 = cur.rearrange("p (t a) -> p t a", a=2)
                nc.vector.tensor_add(
                    out=pt[:, k * nseg : (k + 1) * nseg],
                    in0=c3[:, :, 0:1],
                    in1=c3[:, :, 1:2],
                )

            # software-pipelined emission: chunk k+1's loads/elementwise are
            # emitted (and thus scheduled) before chunk k's ladder, so a
            # ladder stalled on GPSIMD doesn't block the next chunk's DVE work
            pending = None
            for k in range(NCH):
                combo = emit_loads_and_elementwise(k)
                if pending is not None:
                    emit_ladder(k - 1, pending)
                pending = combo
            emit_ladder(NCH - 1, pending)

            # un-interleave and combine: net = port - 0.001 * turn
            pt3 = pt[:].rearrange("p (k d) -> p k d", d=2 * TC)
            nc.vector.scalar_tensor_tensor(
                out=net[:].rearrange("p (k d) -> p k d", d=TC),
                in0=pt3[:, :, TC : 2 * TC],
                scalar=-TRANSACTION_COST,
                in1=pt3[:, :, 0:TC],
                op0=Alu.mult,
                op1=Alu.add,
            )
            # g = 1 + net
            nc.vector.tensor_scalar_add(out=g[:], in0=net[:], scalar1=1.0)
            # equity: eq[0] = 1, eq[1:] = cumprod(g)
            nc.vector.memset(eq[:, 0:1], 1.0)
            nc.vector.tensor_tensor_scan(
                out=eq[:, 1 : T + 1],
                data0=g[:],
                data1=g[:],
                initial=1.0,
                op0=Alu.mult,
                op1=Alu.bypass,
            )

            nc.sync.dma_start(out=net_out[:], in_=net[:])
            nc.sync.dma_start(out=eq_out[:], in_=eq[:])

    nc.compile()
    return nc


def _get_compiled():
    global _compiled
    if _compiled is None:
        _compiled = _build()
    return _compiled


def kernel(allocations, returns):
    global LAST_RESULTS
    from concourse.bass_utils import run_bass_kernel_spmd

    nc = _get_compiled()

    a = np.asarray(allocations, dtype=np.float32).astype(np.float16).reshape(B, T * A)
    r = np.asarray(returns, dtype=np.float32).astype(np.float16).reshape(B, T * A)

    in_maps = [
        {"alloc": a[i * BP : (i + 1) * BP], "ret": r[i * BP : (i + 1) * BP]}
        for i in range(NCORES)
    ]
    res = run_bass_kernel_spmd(nc, in_maps, core_ids=list(range(NCORES)))
    LAST_RESULTS = res

    equity = np.concatenate([res.results[i]["equity"] for i in range(NCORES)], axis=0)
    net = np.concatenate([res.results[i]["net"] for i in range(NCORES)], axis=0)
    return equity, net


# revision 11
# speedup vs baseline: 1.0047x; 1.0035x over previous
"""Trainium2 Bass kernel for DifferentiablePortfolioSim.

Computes, for allocations/returns of shape [B, T, A] = [1024, 2048, 64]:
    port_return[b,t] = sum_a alloc[b,t,a] * ret[b,t,a]
    turnover[b,t]    = sum_a |alloc[b,t,a] - alloc[b,t-1,a]|   (alloc[:,-1]=0)
    net_return       = port_return - 0.001 * turnover
    equity_curve     = [1, cumprod_t(1 + net_return)]          # [B, T+1]
Returns (equity_curve, net_return).

Sharding: data parallel over batch, 128 rows per core on 8 cores; batch rows
on the 128 SBUF partitions, time*assets streamed on the free dim in chunks.

Inputs are pre-cast to fp16 on the host: halves HBM traffic (the memory
roofline) and enables the DVE 2x perf mode for the elementwise passes.
Since equity decays exponentially (mean net return is negative),
absmax-relative error stays ~1e-4.

Engine split per chunk (DVE is the measured bottleneck, ~0.52ns/elem at 2x;
GPSIMD ~3.2ns/elem; ACT ~0.9ns/elem 1-input only):
  - DVE:  fp16 product into the low half of a combo tile, the first
          SUB_DVE timesteps of the shifted diff, and one pairwise-add
          reduction ladder over the combo tile (TensorReduce has no DVE
          perf modes, a ladder of fp16 2x adds is ~2x faster).
  - ACT:  elementwise |diff| into the high half of the combo tile
  - GPSIMD: the remaining timesteps of the shifted diff
The ladder output interleaves port/turn per chunk in one persistent tile;
the tail un-interleaves via strided access patterns.
"""

import numpy as np

B, T, A = 1024, 2048, 64
NCORES = 8
BP = B // NCORES  # 128 batch rows per core == SBUF partitions
TC = 64           # timesteps per chunk
NCH = T // TC
SUB_DVE = 26      # timesteps of the diff pass on DVE; rest on GPSIMD

TRANSACTION_COST = 0.001

_compiled = None
LAST_RESULTS = None


def _build():
    import concourse.mybir as mybir
    from concourse import bacc
    from concourse.tile import TileContext

    f32 = mybir.dt.float32
    f16 = mybir.dt.float16
    Alu = mybir.AluOpType

    nc = bacc.Bacc(
        "TRN2",
        debug=False,
        target_bir_lowering=False,
        num_devices=NCORES,
        dynamic_dma_scratch_size=2048,
    )

    a_in = nc.dram_tensor("alloc", [BP, T * A], f16, kind="ExternalInput").ap()
    r_in = nc.dram_tensor("ret", [BP, T * A], f16, kind="ExternalInput").ap()
    eq_out = nc.dram_tensor("equity", [BP, T + 1], f32, kind="ExternalOutput").ap()
    net_out = nc.dram_tensor("net", [BP, T], f32, kind="ExternalOutput").ap()

    with TileContext(nc) as tc:
        with (
            tc.tile_pool(name="persist", bufs=1) as pp,
            tc.tile_pool(name="dma", bufs=3) as dp,
            tc.tile_pool(name="chunk", bufs=3) as cp,
            tc.tile_pool(name="combop", bufs=4) as cbp,
            tc.tile_pool(name="lvls", bufs=2) as lp,
        ):
            # pt interleaves [port(TC) | turn(TC)] per chunk
            pt = pp.tile([BP, 2 * T], f32, tag="pt")
            net = pp.tile([BP, T], f32, tag="net")
            g = pp.tile([BP, T], f32, tag="g")
            eq = pp.tile([BP, T + 1], f32, tag="eq")

            def emit_loads_and_elementwise(k):
                t0 = k * TC
                # a_t holds TC+1 timesteps: one lookback step + the chunk.
                a_t = dp.tile([BP, (TC + 1) * A], f16, tag="a")
                r_t = dp.tile([BP, TC * A], f16, tag="r")
                dif = cp.tile([BP, TC * A], f16, tag="dif")
                # combo: [ prod (TC*A) | |dif| (TC*A) ]
                combo = cbp.tile([BP, 2 * TC * A], f16, tag="combo")

                if k == 0:
                    # prev_alloc at t=0 is zeros
                    nc.vector.memset(a_t[:, 0:A], 0.0)
                    nc.sync.dma_start(out=a_t[:, A:], in_=a_in[:, 0 : TC * A])
                else:
                    nc.sync.dma_start(
                        out=a_t[:], in_=a_in[:, (t0 - 1) * A : (t0 + TC) * A]
                    )
                nc.sync.dma_start(out=r_t[:], in_=r_in[:, t0 * A : (t0 + TC) * A])

                # GPSIMD's product share first (it's the slowest producer and
                # only needs the DMAs); it feeds the ladder directly so its
                # latency has no ACT hop
                ne = SUB_DVE * A
                nc.gpsimd.tensor_mul(
                    out=combo[:, ne : TC * A],
                    in0=a_t[:, A + ne :],
                    in1=r_t[:, ne:],
                )

                # DVE: full shifted diff early so ACT's abs can start
                nc.vector.tensor_sub(
                    out=dif[:], in0=a_t[:, A:], in1=a_t[:, 0 : TC * A]
                )

                # ACT: |diff| into high half of combo
                nc.scalar.activation(
                    out=combo[:, TC * A :],
                    in_=dif[:],
                    func=mybir.ActivationFunctionType.Abs,
                )

                # DVE's product share
                nc.vector.tensor_mul(
                    out=combo[:, 0:ne], in0=a_t[:, A : A + ne], in1=r_t[:, 0:ne]
                )
                return combo

            def emit_ladder(k, combo):
                # single pairwise-add ladder over both halves:
                # 2*TC segments of length A -> one sum each
                nseg = 2 * TC
                cur = combo[:]
                width = A
                lvl = 0
                while width > 2:
                    width //= 2
                    nxt = lp.tile([BP, nseg * width], f16, tag=f"l{lvl}")
                    c3 = cur.rearrange("p (t a) -> p t a", a=2 * width)
                    nc.vector.tensor_add(
                        out=nxt[:],
                        in0=c3[:, :, 0:width],
                        in1=c3[:, :, width : 2 * width],
                    )
                    cur = nxt[:]
                    lvl += 1
                c3 = cur.rearrange("p (t a) -> p t a", a=2)
                nc.vector.tensor_add(
                    out=pt[:, k * nseg : (k + 1) * nseg],
                    in0=c3[:, :, 0:1],
                    in1=c3[:, :, 1:2],
                )

            # software-pipelined emission: chunk k+1's loads/elementwise are
            # emitted (and thus scheduled) before chunk k's ladder, so a
            # ladder stalled on GPSIMD doesn't block the next chunk's DVE work
            pending = None
            for k in range(NCH):
                combo = emit_loads_and_elementwise(k)
                if pending is not None:
                    emit_ladder(k - 1, pending)
                pending = combo
            emit_ladder(NCH - 1, pending)

            # un-interleave and combine: net = port - 0.001 * turn
            pt3 = pt[:].rearrange("p (k d) -> p k d", d=2 * TC)
            nc.vector.scalar_tensor_tensor(
                out=net[:].rearrange("p (k d) -> p k d", d=TC),
                in0=pt3[:, :, TC : 2 * TC],
                scalar=-TRANSACTION_COST,
                in1=pt3[:, :, 0:TC],
                op0=Alu.mult,
                op1=Alu.add,
            )
            # g = 1 + net
            nc.vector.tensor_scalar_add(out=g[:], in0=net[:], scalar1=1.0)
            # equity: eq[0] = 1, eq[1:] = cumprod(g)
            nc.vector.memset(eq[:, 0:1], 1.0)
            nc.vector.tensor_tensor_scan(
                out=eq[:, 1 : T + 1],
                data0=g[:],
                data1=g[:],
                initial=1.0,
                op0=Alu.mult,
                op1=Alu.bypass,
            )

            nc.sync.dma_start(out=net_out[:], in_=net[:])
            nc.sync.dma_start(out=eq_out[:], in_=eq[:])

    nc.compile()
    return nc


def _get_compiled():
    global _compiled
    if _compiled is None:
        _compiled = _build()
    return _compiled


def kernel(allocations, returns):
    global LAST_RESULTS
    from concourse.bass_utils import run_bass_kernel_spmd

    nc = _get_compiled()

    a = np.asarray(allocations, dtype=np.float32).astype(np.float16).reshape(B, T * A)
    r = np.asarray(returns, dtype=np.float32).astype(np.float16).reshape(B, T * A)

    in_maps = [
        {"alloc": a[i * BP : (i + 1) * BP], "ret": r[i * BP : (i + 1) * BP]}
        for i in range(NCORES)
    ]
    res = run_bass_kernel_spmd(nc, in_maps, core_ids=list(range(NCORES)))
    LAST_RESULTS = res

    equity = np.concatenate([res.results[i]["equity"] for i in range(NCORES)], axis=0)
    net = np.concatenate([res.results[i]["net"] for i in range(NCORES)], axis=0)
    return equity, net


# revision 14
# speedup vs baseline: 1.2266x; 1.2208x over previous
"""Trainium2 Bass kernel for DifferentiablePortfolioSim.

Computes, for allocations/returns of shape [B, T, A] = [1024, 2048, 64]:
    port_return[b,t] = sum_a alloc[b,t,a] * ret[b,t,a]
    turnover[b,t]    = sum_a |alloc[b,t,a] - alloc[b,t-1,a]|   (alloc[:,-1]=0)
    net_return       = port_return - 0.001 * turnover
    equity_curve     = [1, cumprod_t(1 + net_return)]          # [B, T+1]
Returns (equity_curve, net_return).

Sharding: data parallel over batch, 128 rows per core on 8 cores; batch rows
on the 128 SBUF partitions, time*assets streamed on the free dim in chunks.

Inputs are pre-cast to fp16 on the host: halves HBM traffic (the memory
roofline) and enables the DVE 2x perf mode for the elementwise passes.
Since equity decays exponentially (mean net return is negative),
absmax-relative error stays ~1e-4.

Engine split per chunk (DVE is the measured bottleneck, ~0.52ns/elem at 2x;
GPSIMD ~3.2ns/elem; ACT ~0.9ns/elem 1-input only):
  - DVE:  fp16 product into the low half of a combo tile, the first
          SUB_DVE timesteps of the shifted diff, and one pairwise-add
          reduction ladder over the combo tile (TensorReduce has no DVE
          perf modes, a ladder of fp16 2x adds is ~2x faster).
  - ACT:  elementwise |diff| into the high half of the combo tile
  - GPSIMD: the remaining timesteps of the shifted diff
The ladder output interleaves port/turn per chunk in one persistent tile;
the tail un-interleaves via strided access patterns.
"""

import numpy as np

B, T, A = 1024, 2048, 64
NCORES = 8
BP = B // NCORES  # 128 batch rows per core == SBUF partitions
TC = 64           # timesteps per chunk
NCH = T // TC
SUB_DVE = 26      # timesteps of the diff pass on DVE; rest on GPSIMD

TRANSACTION_COST = 0.001

_compiled = None
LAST_RESULTS = None


def _build():
    import concourse.mybir as mybir
    from concourse import bacc
    from concourse.tile import TileContext

    f32 = mybir.dt.float32
    f16 = mybir.dt.float16
    Alu = mybir.AluOpType

    nc = bacc.Bacc(
        "TRN2",
        debug=False,
        target_bir_lowering=False,
        num_devices=NCORES,
        dynamic_dma_scratch_size=2048,
    )

    a_in = nc.dram_tensor("alloc", [BP, T * A], f16, kind="ExternalInput").ap()
    r_in = nc.dram_tensor("ret", [BP, T * A], f16, kind="ExternalInput").ap()
    eq_out = nc.dram_tensor("equity", [BP, T + 1], f32, kind="ExternalOutput").ap()
    net_out = nc.dram_tensor("net", [BP, T], f32, kind="ExternalOutput").ap()

    with TileContext(nc) as tc:
        with (
            tc.tile_pool(name="persist", bufs=1) as pp,
            tc.tile_pool(name="dma", bufs=4) as dp,
            tc.tile_pool(name="chunk", bufs=2) as cp,
            tc.tile_pool(name="combop", bufs=3) as cbp,
            tc.tile_pool(name="lvls", bufs=2) as lp,
            tc.tile_pool(name="gpl", bufs=4) as gp,
        ):
            # pt interleaves [port(TC) | turn(TC)] per chunk
            pt = pp.tile([BP, 2 * T], f32, tag="pt")
            net = pp.tile([BP, T], f32, tag="net")
            g = pp.tile([BP, T], f32, tag="g")
            eq = pp.tile([BP, T + 1], f32, tag="eq")

            def emit_loads_and_elementwise(k):
                t0 = k * TC
                # a_t holds TC+1 timesteps: one lookback step + the chunk.
                a_t = dp.tile([BP, (TC + 1) * A], f16, tag="a")
                r_t = dp.tile([BP, TC * A], f16, tag="r")
                dif = cp.tile([BP, TC * A], f16, tag="dif")
                # combo: [ prod (TC*A) | |dif| (TC*A) ]
                combo = cbp.tile([BP, 2 * TC * A], f16, tag="combo")

                if k == 0:
                    # prev_alloc at t=0 is zeros
                    nc.vector.memset(a_t[:, 0:A], 0.0)
                    nc.sync.dma_start(out=a_t[:, A:], in_=a_in[:, 0 : TC * A])
                else:
                    nc.sync.dma_start(
                        out=a_t[:], in_=a_in[:, (t0 - 1) * A : (t0 + TC) * A]
                    )
                nc.sync.dma_start(out=r_t[:], in_=r_in[:, t0 * A : (t0 + TC) * A])

                # DVE: full shifted diff early so ACT's abs can start
                nc.vector.tensor_sub(
                    out=dif[:], in0=a_t[:, A:], in1=a_t[:, 0 : TC * A]
                )

                # ACT: |diff| into high half of combo
                nc.scalar.activation(
                    out=combo[:, TC * A :],
                    in_=dif[:],
                    func=mybir.ActivationFunctionType.Abs,
                )

                # DVE: full fp16 product (2x mode). Everything that feeds the
                # DVE ladder stays on DVE/ACT so the in-order engines never
                # wait on the slow GPSIMD (which caused convoy stalls).
                nc.vector.tensor_mul(
                    out=combo[:, 0 : TC * A], in0=a_t[:, A:], in1=r_t[:]
                )
                return combo

            def emit_ladder(k, combo):
                # single pairwise-add ladder over both halves:
                # 2*TC segments of length A -> one sum each.
                # Levels 64->32->16->8 on DVE (2x fp16); the 8->4->2->1 tail
                # runs on GPSIMD as a pure sink: nothing downstream waits on
                # it until the end-of-kernel combine, so its slowness and
                # jitter stay off the critical path.
                nseg = 2 * TC
                cur = combo[:]
                width = A
                lvl = 0
                while width > 8:
                    width //= 2
                    nxt = lp.tile([BP, nseg * width], f16, tag=f"l{lvl}")
                    c3 = cur.rearrange("p (t a) -> p t a", a=2 * width)
                    nc.vector.tensor_add(
                        out=nxt[:],
                        in0=c3[:, :, 0:width],
                        in1=c3[:, :, width : 2 * width],
                    )
                    cur = nxt[:]
                    lvl += 1
                while width > 2:
                    width //= 2
                    nxt = gp.tile([BP, nseg * width], f16, tag=f"g{lvl}")
                    c3 = cur.rearrange("p (t a) -> p t a", a=2 * width)
                    nc.gpsimd.tensor_add(
                        out=nxt[:],
                        in0=c3[:, :, 0:width],
                        in1=c3[:, :, width : 2 * width],
                    )
                    cur = nxt[:]
                    lvl += 1
                c3 = cur.rearrange("p (t a) -> p t a", a=2)
                nc.gpsimd.tensor_add(
                    out=pt[:, k * nseg : (k + 1) * nseg],
                    in0=c3[:, :, 0:1],
                    in1=c3[:, :, 1:2],
                )

            # software-pipelined emission: chunk k+1's loads/elementwise are
            # emitted (and thus scheduled) before chunk k's ladder, so a
            # ladder stalled on GPSIMD doesn't block the next chunk's DVE work
            pending = None
            for k in range(NCH):
                combo = emit_loads_and_elementwise(k)
                if pending is not None:
                    emit_ladder(k - 1, pending)
                pending = combo
            emit_ladder(NCH - 1, pending)

            # un-interleave and combine: net = port - 0.001 * turn
            pt3 = pt[:].rearrange("p (k d) -> p k d", d=2 * TC)
            nc.vector.scalar_tensor_tensor(
                out=net[:].rearrange("p (k d) -> p k d", d=TC),
                in0=pt3[:, :, TC : 2 * TC],
                scalar=-TRANSACTION_COST,
                in1=pt3[:, :, 0:TC],
                op0=Alu.mult,
                op1=Alu.add,
            )
            # g = 1 + net
            nc.vector.tensor_scalar_add(out=g[:], in0=net[:], scalar1=1.0)
            # equity: eq[0] = 1, eq[1:] = cumprod(g)
            nc.vector.memset(eq[:, 0:1], 1.0)
            nc.vector.tensor_tensor_scan(
                out=eq[:, 1 : T + 1],
                data0=g[:],
                data1=g[:],
                initial=1.0,
                op0=Alu.mult,
                op1=Alu.bypass,
            )

            nc.sync.dma_start(out=net_out[:], in_=net[:])
            nc.sync.dma_start(out=eq_out[:], in_=eq[:])

    nc.compile()
    return nc


def _get_compiled():
    global _compiled
    if _compiled is None:
        _compiled = _build()
    return _compiled


def kernel(allocations, returns):
    global LAST_RESULTS
    from concourse.bass_utils import run_bass_kernel_spmd

    nc = _get_compiled()

    a = np.asarray(allocations, dtype=np.float32).astype(np.float16).reshape(B, T * A)
    r = np.asarray(returns, dtype=np.float32).astype(np.float16).reshape(B, T * A)

    in_maps = [
        {"alloc": a[i * BP : (i + 1) * BP], "ret": r[i * BP : (i + 1) * BP]}
        for i in range(NCORES)
    ]
    res = run_bass_kernel_spmd(nc, in_maps, core_ids=list(range(NCORES)))
    LAST_RESULTS = res

    equity = np.concatenate([res.results[i]["equity"] for i in range(NCORES)], axis=0)
    net = np.concatenate([res.results[i]["net"] for i in range(NCORES)], axis=0)
    return equity, net


# revision 16
# speedup vs baseline: 1.2422x; 1.0127x over previous
"""Trainium2 Bass kernel for DifferentiablePortfolioSim.

Computes, for allocations/returns of shape [B, T, A] = [1024, 2048, 64]:
    port_return[b,t] = sum_a alloc[b,t,a] * ret[b,t,a]
    turnover[b,t]    = sum_a |alloc[b,t,a] - alloc[b,t-1,a]|   (alloc[:,-1]=0)
    net_return       = port_return - 0.001 * turnover
    equity_curve     = [1, cumprod_t(1 + net_return)]          # [B, T+1]
Returns (equity_curve, net_return).

Sharding: data parallel over batch, 128 rows per core on 8 cores; batch rows
on the 128 SBUF partitions, time*assets streamed on the free dim in chunks.

Inputs are pre-cast to fp16 on the host: halves HBM traffic (the memory
roofline) and enables the DVE 2x perf mode for the elementwise passes.
Since equity decays exponentially (mean net return is negative),
absmax-relative error stays ~1e-4.

Engine split per chunk (DVE is the measured bottleneck, ~0.52ns/elem at 2x;
GPSIMD ~3.2ns/elem; ACT ~0.9ns/elem 1-input only):
  - DVE:  fp16 product into the low half of a combo tile, the first
          SUB_DVE timesteps of the shifted diff, and one pairwise-add
          reduction ladder over the combo tile (TensorReduce has no DVE
          perf modes, a ladder of fp16 2x adds is ~2x faster).
  - ACT:  elementwise |diff| into the high half of the combo tile
  - GPSIMD: the remaining timesteps of the shifted diff
The ladder output interleaves port/turn per chunk in one persistent tile;
the tail un-interleaves via strided access patterns.
"""

import numpy as np

B, T, A = 1024, 2048, 64
NCORES = 8
BP = B // NCORES  # 128 batch rows per core == SBUF partitions
TC = 64           # timesteps per chunk
NCH = T // TC
SUB_DVE = 26      # timesteps of the diff pass on DVE; rest on GPSIMD

TRANSACTION_COST = 0.001

_compiled = None
LAST_RESULTS = None


def _build():
    import concourse.mybir as mybir
    from concourse import bacc
    from concourse.tile import TileContext

    f32 = mybir.dt.float32
    f16 = mybir.dt.float16
    Alu = mybir.AluOpType

    nc = bacc.Bacc(
        "TRN2",
        debug=False,
        target_bir_lowering=False,
        num_devices=NCORES,
        dynamic_dma_scratch_size=2048,
    )

    a_in = nc.dram_tensor("alloc", [BP, T * A], f16, kind="ExternalInput").ap()
    r_in = nc.dram_tensor("ret", [BP, T * A], f16, kind="ExternalInput").ap()
    eq_out = nc.dram_tensor("equity", [BP, T + 1], f32, kind="ExternalOutput").ap()
    net_out = nc.dram_tensor("net", [BP, T], f32, kind="ExternalOutput").ap()

    with TileContext(nc) as tc:
        with (
            tc.tile_pool(name="persist", bufs=1) as pp,
            tc.tile_pool(name="dma", bufs=3) as dp,
            tc.tile_pool(name="chunk", bufs=2) as cp,
            tc.tile_pool(name="combop", bufs=4) as cbp,
            tc.tile_pool(name="lvls", bufs=2) as lp,
            tc.tile_pool(name="gpl", bufs=4) as gp,
        ):
            # pt interleaves [port(TC) | turn(TC)] per chunk
            pt = pp.tile([BP, 2 * T], f32, tag="pt")
            net = pp.tile([BP, T], f32, tag="net")
            g = pp.tile([BP, T], f32, tag="g")
            eq = pp.tile([BP, T + 1], f32, tag="eq")

            def emit_loads_and_elementwise(k):
                t0 = k * TC
                # a_t holds TC+1 timesteps: one lookback step + the chunk.
                a_t = dp.tile([BP, (TC + 1) * A], f16, tag="a")
                r_t = dp.tile([BP, TC * A], f16, tag="r")
                dif = cp.tile([BP, TC * A], f16, tag="dif")
                # combo: [ prod (TC*A) | |dif| (TC*A) ]
                combo = cbp.tile([BP, 2 * TC * A], f16, tag="combo")

                if k == 0:
                    # prev_alloc at t=0 is zeros
                    nc.vector.memset(a_t[:, 0:A], 0.0)
                    nc.sync.dma_start(out=a_t[:, A:], in_=a_in[:, 0 : TC * A])
                else:
                    nc.sync.dma_start(
                        out=a_t[:], in_=a_in[:, (t0 - 1) * A : (t0 + TC) * A]
                    )
                nc.sync.dma_start(out=r_t[:], in_=r_in[:, t0 * A : (t0 + TC) * A])

                # DVE: full fp16 product (2x mode) first - it only needs the
                # DMAs. Everything that feeds the DVE ladder stays on DVE/ACT
                # so the in-order engines never wait on the slow GPSIMD
                # (which caused convoy stalls).
                nc.vector.tensor_mul(
                    out=combo[:, 0 : TC * A], in0=a_t[:, A:], in1=r_t[:]
                )

                # DVE: full shifted diff, then ACT's abs
                nc.vector.tensor_sub(
                    out=dif[:], in0=a_t[:, A:], in1=a_t[:, 0 : TC * A]
                )
                nc.scalar.activation(
                    out=combo[:, TC * A :],
                    in_=dif[:],
                    func=mybir.ActivationFunctionType.Abs,
                )
                return combo

            def emit_ladder(k, combo):
                # single pairwise-add ladder over both halves:
                # 2*TC segments of length A -> one sum each.
                # Levels 64->32->16->8 on DVE (2x fp16); the 8->4->2->1 tail
                # runs on GPSIMD as a pure sink: nothing downstream waits on
                # it until the end-of-kernel combine, so its slowness and
                # jitter stay off the critical path.
                nseg = 2 * TC
                cur = combo[:]
                width = A
                lvl = 0
                while width > 8:
                    width //= 2
                    nxt = lp.tile([BP, nseg * width], f16, tag=f"l{lvl}")
                    c3 = cur.rearrange("p (t a) -> p t a", a=2 * width)
                    nc.vector.tensor_add(
                        out=nxt[:],
                        in0=c3[:, :, 0:width],
                        in1=c3[:, :, width : 2 * width],
                    )
                    cur = nxt[:]
                    lvl += 1
                while width > 2:
                    width //= 2
                    nxt = gp.tile([BP, nseg * width], f16, tag=f"g{lvl}")
                    c3 = cur.rearrange("p (t a) -> p t a", a=2 * width)
                    nc.gpsimd.tensor_add(
                        out=nxt[:],
                        in0=c3[:, :, 0:width],
                        in1=c3[:, :, width : 2 * width],
                    )
                    cur = nxt[:]
                    lvl += 1
                c3 = cur.rearrange("p (t a) -> p t a", a=2)
                nc.gpsimd.tensor_add(
                    out=pt[:, k * nseg : (k + 1) * nseg],
                    in0=c3[:, :, 0:1],
                    in1=c3[:, :, 1:2],
                )

            # software-pipelined emission: chunk k+1's loads/elementwise are
            # emitted (and thus scheduled) before chunk k's ladder, so a
            # ladder stalled on GPSIMD doesn't block the next chunk's DVE work
            pending = None
            for k in range(NCH):
                combo = emit_loads_and_elementwise(k)
                if pending is not None:
                    emit_ladder(k - 1, pending)
                pending = combo
            emit_ladder(NCH - 1, pending)

            # un-interleave and combine: net = port - 0.001 * turn
            pt3 = pt[:].rearrange("p (k d) -> p k d", d=2 * TC)
            nc.vector.scalar_tensor_tensor(
                out=net[:].rearrange("p (k d) -> p k d", d=TC),
                in0=pt3[:, :, TC : 2 * TC],
                scalar=-TRANSACTION_COST,
                in1=pt3[:, :, 0:TC],
                op0=Alu.mult,
                op1=Alu.add,
            )
            # g = 1 + net
            nc.vector.tensor_scalar_add(out=g[:], in0=net[:], scalar1=1.0)
            # equity: eq[0] = 1, eq[1:] = cumprod(g)
            nc.vector.memset(eq[:, 0:1], 1.0)
            nc.vector.tensor_tensor_scan(
                out=eq[:, 1 : T + 1],
                data0=g[:],
                data1=g[:],
                initial=1.0,
                op0=Alu.mult,
                op1=Alu.bypass,
            )

            nc.sync.dma_start(out=net_out[:], in_=net[:])
            nc.sync.dma_start(out=eq_out[:], in_=eq[:])

    nc.compile()
    return nc


def _get_compiled():
    global _compiled
    if _compiled is None:
        _compiled = _build()
    return _compiled


def kernel(allocations, returns):
    global LAST_RESULTS
    from concourse.bass_utils import run_bass_kernel_spmd

    nc = _get_compiled()

    a = np.asarray(allocations, dtype=np.float32).astype(np.float16).reshape(B, T * A)
    r = np.asarray(returns, dtype=np.float32).astype(np.float16).reshape(B, T * A)

    in_maps = [
        {"alloc": a[i * BP : (i + 1) * BP], "ret": r[i * BP : (i + 1) * BP]}
        for i in range(NCORES)
    ]
    res = run_bass_kernel_spmd(nc, in_maps, core_ids=list(range(NCORES)))
    LAST_RESULTS = res

    equity = np.concatenate([res.results[i]["equity"] for i in range(NCORES)], axis=0)
    net = np.concatenate([res.results[i]["net"] for i in range(NCORES)], axis=0)
    return equity, net


# revision 17
# speedup vs baseline: 1.2453x; 1.0025x over previous
"""Trainium2 Bass kernel for DifferentiablePortfolioSim.

Computes, for allocations/returns of shape [B, T, A] = [1024, 2048, 64]:
    port_return[b,t] = sum_a alloc[b,t,a] * ret[b,t,a]
    turnover[b,t]    = sum_a |alloc[b,t,a] - alloc[b,t-1,a]|   (alloc[:,-1]=0)
    net_return       = port_return - 0.001 * turnover
    equity_curve     = [1, cumprod_t(1 + net_return)]          # [B, T+1]
Returns (equity_curve, net_return).

Sharding: data parallel over batch, 128 rows per core on 8 cores; batch rows
on the 128 SBUF partitions, time*assets streamed on the free dim in chunks.

Inputs are pre-cast to fp16 on the host: halves HBM traffic (the memory
roofline) and enables the DVE 2x perf mode for the elementwise passes.
Since equity decays exponentially (mean net return is negative),
absmax-relative error stays ~1e-4.

Engine split per chunk (DVE is the measured bottleneck, ~0.52ns/elem at 2x;
GPSIMD ~3.2ns/elem; ACT ~0.9ns/elem 1-input only):
  - DVE:  fp16 product into the low half of a combo tile, the first
          SUB_DVE timesteps of the shifted diff, and one pairwise-add
          reduction ladder over the combo tile (TensorReduce has no DVE
          perf modes, a ladder of fp16 2x adds is ~2x faster).
  - ACT:  elementwise |diff| into the high half of the combo tile
  - GPSIMD: the remaining timesteps of the shifted diff
The ladder output interleaves port/turn per chunk in one persistent tile;
the tail un-interleaves via strided access patterns.
"""

import numpy as np

B, T, A = 1024, 2048, 64
NCORES = 8
BP = B // NCORES  # 128 batch rows per core == SBUF partitions
TC = 64           # timesteps per chunk
NCH = T // TC
SUB_DVE = 26      # timesteps of the diff pass on DVE; rest on GPSIMD

TRANSACTION_COST = 0.001

_compiled = None
LAST_RESULTS = None


def _build():
    import concourse.mybir as mybir
    from concourse import bacc
    from concourse.tile import TileContext

    f32 = mybir.dt.float32
    f16 = mybir.dt.float16
    Alu = mybir.AluOpType

    nc = bacc.Bacc(
        "TRN2",
        debug=False,
        target_bir_lowering=False,
        num_devices=NCORES,
        dynamic_dma_scratch_size=2048,
    )

    a_in = nc.dram_tensor("alloc", [BP, T * A], f16, kind="ExternalInput").ap()
    r_in = nc.dram_tensor("ret", [BP, T * A], f16, kind="ExternalInput").ap()
    eq_out = nc.dram_tensor("equity", [BP, T + 1], f32, kind="ExternalOutput").ap()
    net_out = nc.dram_tensor("net", [BP, T], f32, kind="ExternalOutput").ap()

    with TileContext(nc) as tc:
        with (
            tc.tile_pool(name="persist", bufs=1) as pp,
            tc.tile_pool(name="dma", bufs=4) as dp,
            tc.tile_pool(name="chunk", bufs=3) as cp,
            tc.tile_pool(name="combop", bufs=3) as cbp,
            tc.tile_pool(name="l01", bufs=3) as lp,
            tc.tile_pool(name="l2p", bufs=2) as l2p,
            tc.tile_pool(name="gpl", bufs=3) as gp,
        ):
            # pt interleaves [port(TC) | turn(TC)] per chunk
            pt = pp.tile([BP, 2 * T], f32, tag="pt")
            net = pp.tile([BP, T], f32, tag="net")
            eq = pp.tile([BP, T + 1], f32, tag="eq")

            def emit_loads_and_elementwise(k):
                t0 = k * TC
                # a_t holds TC+1 timesteps: one lookback step + the chunk.
                a_t = dp.tile([BP, (TC + 1) * A], f16, tag="a")
                r_t = dp.tile([BP, TC * A], f16, tag="r")
                dif = cp.tile([BP, TC * A], f16, tag="dif")
                # combo: [ prod (TC*A) | |dif| (TC*A) ]
                combo = cbp.tile([BP, 2 * TC * A], f16, tag="combo")

                if k == 0:
                    # prev_alloc at t=0 is zeros
                    nc.vector.memset(a_t[:, 0:A], 0.0)
                    nc.sync.dma_start(out=a_t[:, A:], in_=a_in[:, 0 : TC * A])
                else:
                    nc.sync.dma_start(
                        out=a_t[:], in_=a_in[:, (t0 - 1) * A : (t0 + TC) * A]
                    )
                nc.sync.dma_start(out=r_t[:], in_=r_in[:, t0 * A : (t0 + TC) * A])

                # DVE: full fp16 product (2x mode) first - it only needs the
                # DMAs. Everything that feeds the DVE ladder stays on DVE/ACT
                # so the in-order engines never wait on the slow GPSIMD
                # (which caused convoy stalls).
                nc.vector.tensor_mul(
                    out=combo[:, 0 : TC * A], in0=a_t[:, A:], in1=r_t[:]
                )

                # DVE: full shifted diff, then ACT's abs
                nc.vector.tensor_sub(
                    out=dif[:], in0=a_t[:, A:], in1=a_t[:, 0 : TC * A]
                )
                nc.scalar.activation(
                    out=combo[:, TC * A :],
                    in_=dif[:],
                    func=mybir.ActivationFunctionType.Abs,
                )
                return combo

            def emit_ladder(k, combo):
                # single pairwise-add ladder over both halves:
                # 2*TC segments of length A -> one sum each.
                # Levels 64->32->16->8 on DVE (2x fp16); the 8->4->2->1 tail
                # runs on GPSIMD as a pure sink: nothing downstream waits on
                # it until the end-of-kernel combine, so its slowness and
                # jitter stay off the critical path.
                nseg = 2 * TC
                cur = combo[:]
                width = A
                lvl = 0
                while width > 8:
                    width //= 2
                    pool = lp if width > 8 else l2p
                    nxt = pool.tile([BP, nseg * width], f16, tag=f"l{lvl}")
                    c3 = cur.rearrange("p (t a) -> p t a", a=2 * width)
                    nc.vector.tensor_add(
                        out=nxt[:],
                        in0=c3[:, :, 0:width],
                        in1=c3[:, :, width : 2 * width],
                    )
                    cur = nxt[:]
                    lvl += 1
                while width > 2:
                    width //= 2
                    nxt = gp.tile([BP, nseg * width], f16, tag=f"g{lvl}")
                    c3 = cur.rearrange("p (t a) -> p t a", a=2 * width)
                    nc.gpsimd.tensor_add(
                        out=nxt[:],
                        in0=c3[:, :, 0:width],
                        in1=c3[:, :, width : 2 * width],
                    )
                    cur = nxt[:]
                    lvl += 1
                c3 = cur.rearrange("p (t a) -> p t a", a=2)
                nc.gpsimd.tensor_add(
                    out=pt[:, k * nseg : (k + 1) * nseg],
                    in0=c3[:, :, 0:1],
                    in1=c3[:, :, 1:2],
                )

            # software-pipelined emission: chunk k+1's loads/elementwise are
            # emitted (and thus scheduled) before chunk k's ladder, so a
            # ladder stalled on GPSIMD doesn't block the next chunk's DVE work
            pending = None
            for k in range(NCH):
                combo = emit_loads_and_elementwise(k)
                if pending is not None:
                    emit_ladder(k - 1, pending)
                pending = combo
            emit_ladder(NCH - 1, pending)

            # un-interleave and combine: net = port - 0.001 * turn
            pt3 = pt[:].rearrange("p (k d) -> p k d", d=2 * TC)
            nc.vector.scalar_tensor_tensor(
                out=net[:].rearrange("p (k d) -> p k d", d=TC),
                in0=pt3[:, :, TC : 2 * TC],
                scalar=-TRANSACTION_COST,
                in1=pt3[:, :, 0:TC],
                op0=Alu.mult,
                op1=Alu.add,
            )
            # g = 1 + net (transient: reuse a ladder slot)
            g = lp.tile([BP, T], f32, tag="l0")
            nc.vector.tensor_scalar_add(out=g[:], in0=net[:], scalar1=1.0)
            # equity: eq[0] = 1, eq[1:] = cumprod(g)
            nc.vector.memset(eq[:, 0:1], 1.0)
            nc.vector.tensor_tensor_scan(
                out=eq[:, 1 : T + 1],
                data0=g[:],
                data1=g[:],
                initial=1.0,
                op0=Alu.mult,
                op1=Alu.bypass,
            )

            nc.sync.dma_start(out=net_out[:], in_=net[:])
            nc.sync.dma_start(out=eq_out[:], in_=eq[:])

    nc.compile()
    return nc


def _get_compiled():
    global _compiled
    if _compiled is None:
        _compiled = _build()
    return _compiled


def kernel(allocations, returns):
    global LAST_RESULTS
    from concourse.bass_utils import run_bass_kernel_spmd

    nc = _get_compiled()

    a = np.asarray(allocations, dtype=np.float32).astype(np.float16).reshape(B, T * A)
    r = np.asarray(returns, dtype=np.float32).astype(np.float16).reshape(B, T * A)

    in_maps = [
        {"alloc": a[i * BP : (i + 1) * BP], "ret": r[i * BP : (i + 1) * BP]}
        for i in range(NCORES)
    ]
    res = run_bass_kernel_spmd(nc, in_maps, core_ids=list(range(NCORES)))
    LAST_RESULTS = res

    equity = np.concatenate([res.results[i]["equity"] for i in range(NCORES)], axis=0)
    net = np.concatenate([res.results[i]["net"] for i in range(NCORES)], axis=0)
    return equity, net
